# revision 2
# baseline (speedup 1.0000x reference)
"""Trainium2 Bass kernel for nn_ComputeCorr (retrieval_knn).

Math: for each batch pair b:
  d[n,m] = ||sf[n]-tf[m]||^2,  sf = src_f[b].T, tf = tgt_f[b].T   (D=64 features)
  src_corr[b] = softmax_m(-d) @ tgt[b];  tgt_corr[b] = softmax_n(-d.T) @ src[b]

Key restructure (per side, shown for src_corr):
  softmax_m(-d)[n,:] @ tgt = (sum_m U[m,n] * [tgt|1][m,:])[:3] / (...)[3]
  with U[m,n] = exp(c0 - d[n,m]) computed directly in [m(partition), n(free)]
  layout:   exponent = 2*(ab[m,n] + 1*(-aa[n]/2)) + (c0 - bb[m])
  i.e. a K=65 matmul (features augmented with a ones row / -aa/2 row) whose
  per-column shift makes the exponent == c0 - d <= c0: no max pass, no
  overflow.  The per-partition bias (c0 - bb[m]) rides the ScalarE
  activation.  Numerator and denominator both come from one PE matmul
  against [tgt|1]; the only non-PE/ACT work is a tiny 4-wide transpose +
  divide at the end.

Sharding: 8 cores = 4 batches x 2 halves. Core c handles batch c//2 and
rows [h*2048,(h+1)*2048) of BOTH outputs (h = c%2). Each side's score
matrix chunk is [4096 x 2048], never materialized in DRAM.
"""

import os
import sys

import numpy as np

for _p in ("/opt/trn_rl_repo", "/root/.axon_site/_ro/trn_rl_repo"):
    if os.path.isdir(_p) and _p not in sys.path:
        sys.path.insert(0, _p)

import concourse.bacc as bacc
import concourse.tile as tile
from concourse import mybir
from concourse.bass_utils import run_bass_kernel_spmd
from concourse.masks import make_identity

B, N, M, D = 4, 4096, 4096, 64
H = N // 2  # rows per core per side
NCORES = 8
C0 = 40.0
KA = D + 1  # augmented contraction dim
MB = 128  # score-partition (m) block
NB = 512  # score-free (n) block
NMB = M // MB  # 32 m blocks
NNB = H // NB  # 4 n blocks per core
F32 = mybir.dt.float32

_PROG = None


def _build_side(nc, tc, pools, identity, lhs_d, rhs_d, bias_d, v_d, out_d):
    big, small, upool, spool, wpool, tpool, epool = pools

    lhs_sb = big.tile([KA, M], F32, tag="lhs")
    rhs_sb = big.tile([KA, H], F32, tag="rhs")
    bias_sb = small.tile([MB, NMB], F32, tag="bias")
    v_sb = small.tile([MB, NMB, 4], F32, tag="v")
    nc.sync.dma_start(out=lhs_sb, in_=lhs_d)
    nc.sync.dma_start(out=rhs_sb, in_=rhs_d)
    nc.sync.dma_start(out=bias_sb, in_=bias_d)
    nc.sync.dma_start(out=v_sb, in_=v_d.rearrange("p (c f) -> p c f", f=4))

    w = [
        wpool.tile([4, NB], F32, tag=f"w{j}", name=f"w{j}") for j in range(NNB)
    ]
    for mi in range(NMB):
        for nj in range(NNB):
            s_ps = spool.tile([MB, NB], F32, tag="s")
            nc.tensor.matmul(
                s_ps,
                lhsT=lhs_sb[:, mi * MB : (mi + 1) * MB],
                rhs=rhs_sb[:, nj * NB : (nj + 1) * NB],
                start=True,
                stop=True,
            )
            u_sb = upool.tile([MB, NB], F32, tag="u")
            nc.scalar.activation(
                out=u_sb,
                in_=s_ps,
                func=mybir.ActivationFunctionType.Exp,
                bias=bias_sb[:, mi : mi + 1],
                scale=2.0,
            )
            nc.tensor.matmul(
                w[nj],
                lhsT=v_sb[:, mi, :],
                rhs=u_sb,
                start=(mi == 0),
                stop=(mi == NMB - 1),
            )

    for nj in range(NNB):
        w_sb = epool.tile([4, NB], F32, tag="wsb")
        nc.scalar.copy(w_sb, w[nj])
        for j2 in range(NB // MB):
            wt_ps = tpool.tile([MB, 4], F32, tag="wt")
            nc.tensor.transpose(wt_ps, w_sb[:, j2 * MB : (j2 + 1) * MB], identity[:4, :4])
            wt_sb = epool.tile([MB, 4], F32, tag="wtsb")
            nc.vector.tensor_copy(wt_sb, wt_ps)
            r_sb = epool.tile([MB, 1], F32, tag="r")
            nc.vector.reciprocal(r_sb, wt_sb[:, 3:4])
            o_sb = epool.tile([MB, 3], F32, tag="o")
            nc.vector.tensor_scalar_mul(o_sb, wt_sb[:, 0:3], r_sb)
            row = nj * NB + j2 * MB
            nc.sync.dma_start(out=out_d[row : row + MB, :], in_=o_sb)


def _build():
    nc = bacc.Bacc("TRN2", target_bir_lowering=False, debug=False)

    def din(name, shape):
        return nc.dram_tensor(name, shape, F32, kind="ExternalInput").ap()

    lhsA = din("lhsA", [KA, M])
    rhsA = din("rhsA", [KA, H])
    biasA = din("biasA", [MB, NMB])
    vA = din("vA", [MB, NMB * 4])
    lhsB = din("lhsB", [KA, N])
    rhsB = din("rhsB", [KA, H])
    biasB = din("biasB", [MB, NMB])
    vB = din("vB", [MB, NMB * 4])
    out_src = nc.dram_tensor("out_src", [H, 3], F32, kind="ExternalOutput").ap()
    out_tgt = nc.dram_tensor("out_tgt", [H, 3], F32, kind="ExternalOutput").ap()

    with tile.TileContext(nc) as tc:
        with (
            tc.tile_pool(name="big", bufs=2) as big,
            tc.tile_pool(name="small", bufs=2) as small,
            tc.tile_pool(name="upool", bufs=4) as upool,
            tc.tile_pool(name="spool", bufs=3, space="PSUM") as spool,
            tc.tile_pool(name="wpool", bufs=1, space="PSUM") as wpool,
            tc.tile_pool(name="tpool", bufs=1, space="PSUM") as tpool,
            tc.tile_pool(name="epool", bufs=2) as epool,
            tc.tile_pool(name="ident", bufs=1) as ident,
        ):
            identity = ident.tile([MB, MB], F32, tag="identity")
            make_identity(nc, identity[:])
            pools = (big, small, upool, spool, wpool, tpool, epool)
            _build_side(nc, tc, pools, identity, lhsA, rhsA, biasA, vA, out_src)
            _build_side(nc, tc, pools, identity, lhsB, rhsB, biasB, vB, out_tgt)

    nc.compile()
    return nc


def _prep_inputs(src, tgt, src_f, tgt_f):
    """Build the 8 per-core input maps (host-side sharding + layout prep)."""
    src = np.ascontiguousarray(src, dtype=np.float32)
    tgt = np.ascontiguousarray(tgt, dtype=np.float32)
    src_f = np.ascontiguousarray(src_f, dtype=np.float32)
    tgt_f = np.ascontiguousarray(tgt_f, dtype=np.float32)
    aa = (src_f * src_f).sum(axis=1)  # [B, N]
    bb = (tgt_f * tgt_f).sum(axis=1)  # [B, M]
    ones_row = np.ones((1, M), np.float32)

    def chunk_cols(x):  # [L] -> [MB, L//MB] with col c, part p <-> x[c*MB+p]
        return np.ascontiguousarray(x.reshape(-1, MB).T)

    def chunk_v(pts):  # [L, 3] -> [MB, (L//MB)*4]
        v = np.concatenate([pts, np.ones((pts.shape[0], 1), np.float32)], axis=1)
        return np.ascontiguousarray(
            v.reshape(-1, MB, 4).transpose(1, 0, 2).reshape(MB, -1)
        )

    in_maps = []
    for c in range(NCORES):
        b, h = divmod(c, 2)
        sl = slice(h * H, (h + 1) * H)
        in_maps.append(
            {
                "lhsA": np.ascontiguousarray(np.vstack([tgt_f[b], ones_row])),
                "rhsA": np.ascontiguousarray(
                    np.vstack([src_f[b][:, sl], (-0.5 * aa[b][sl])[None, :]])
                ),
                "biasA": chunk_cols(C0 - bb[b]),
                "vA": chunk_v(tgt[b]),
                "lhsB": np.ascontiguousarray(np.vstack([src_f[b], ones_row])),
                "rhsB": np.ascontiguousarray(
                    np.vstack([tgt_f[b][:, sl], (-0.5 * bb[b][sl])[None, :]])
                ),
                "biasB": chunk_cols(C0 - aa[b]),
                "vB": chunk_v(src[b]),
            }
        )
    return in_maps


def run(inputs, trace=False, **kw):
    global _PROG
    if _PROG is None:
        _PROG = _build()
    in_maps = _prep_inputs(
        inputs["src"], inputs["tgt"], inputs["src_f"], inputs["tgt_f"]
    )
    bkr = run_bass_kernel_spmd(
        _PROG, in_maps, core_ids=list(range(NCORES)), trace=trace, **kw
    )
    src_corr = np.zeros((B, N, 3), np.float32)
    tgt_corr = np.zeros((B, M, 3), np.float32)
    for c in range(NCORES):
        b, h = divmod(c, 2)
        sl = slice(h * H, (h + 1) * H)
        src_corr[b, sl] = bkr.results[c]["out_src"]
        tgt_corr[b, sl] = bkr.results[c]["out_tgt"]
    return (src_corr, tgt_corr), bkr


def kernel(**inputs):
    out, _ = run(inputs)
    return out


# revision 3
# speedup vs baseline: 1.7971x; 1.7971x over previous
"""Trainium2 Bass kernel for nn_ComputeCorr (retrieval_knn).

Math (per batch pair b, D=64 features):
  d[n,m] = ||sf[n]-tf[m]||^2,  sf = src_f[b].T, tf = tgt_f[b].T
  src_corr[b] = softmax_m(-d) @ tgt[b];  tgt_corr[b] = softmax_n(-d.T) @ src[b]

Restructure (per side, shown for src_corr):
  softmax_m(-d)[n,:] @ tgt = (sum_m U[m,n] * [tgt|1][m,:])[:3] / (...)[3]
  U[m,n] = exp(c0 - d[n,m]) computed directly in [m(part), n(free)] layout.
  The whole exponent (2*ab - aa[n] - bb[m] + c0) is produced by TWO bf16
  matmuls with K-packed augmentation rows (exponent == c0 - d <= c0, so no
  max pass and no overflow):
    MM1 (K=128): [hi(L); lo(L)]^T @ [hi(R); hi(R)]      -> hiL*hiR + loL*hiR
    MM2 (K=67):  [hi(L); 1; bhi; blo]^T @ [lo(R); -|r|^2/2; 1; 1]
                 -> hiL*loR - aa[n]/2 + (c0-bb[m])/2
  (hi/lo = bf16 value-split of the fp32 features; dropped loL*loR term is
  ~2^-16 relative. The -aa[n]/2 row's bf16 rounding is a per-column shift
  that cancels in the softmax normalization.)
  ScalarE then does a bias-free exp (scale=2.0) straight out of PSUM over
  1024-wide tiles, emitting U in bf16; one more bf16 matmul against
  [tgt|1] accumulates numerator+denominator [4, n] in PSUM over all 32
  m-chunks.  Epilogue: tiny PE transpose to [n, 4], reciprocal + multiply
  on DVE, DMA out.

Sharding: 8 cores = 4 batches x 2 halves; core c = batch c//2, rows
[h*2048,(h+1)*2048) of BOTH outputs (h=c%2). The [4096 x 2048] score
block per side is never materialized in DRAM.
"""

import os
import sys

import numpy as np

for _p in ("/opt/trn_rl_repo", "/root/.axon_site/_ro/trn_rl_repo"):
    if os.path.isdir(_p) and _p not in sys.path:
        sys.path.insert(0, _p)

import ml_dtypes

import concourse.bacc as bacc
import concourse.tile as tile
from concourse import mybir
from concourse.bass_utils import run_bass_kernel_spmd
from concourse.masks import make_identity

B, N, M, D = 4, 4096, 4096, 64
H = N // 2  # rows per core per side
NCORES = 8
C0 = 40.0
K2 = D + 3  # MM2 contraction: lo features + shift row + 2 bias rows
MB = 128  # m block (score partition dim)
NB = 512  # n block (matmul free dim)
WACT = 1024  # ACT tile width (2 PSUM banks)
NMB = M // MB  # 32 m blocks
NNB = H // NB  # 4 n blocks per core
F32 = mybir.dt.float32
BF16 = mybir.dt.bfloat16
NPBF = ml_dtypes.bfloat16

_PROG = None


def _build_side(nc, pools, identity, names, out_d):
    big, upool, spool, wpool, epool = pools

    lhs1 = big.tile([2 * D, M], BF16, tag="lhs1", name="lhs1")
    lhs2 = big.tile([K2, M], BF16, tag="lhs2", name="lhs2")
    rhs1 = big.tile([2 * D, H], BF16, tag="rhs1", name="rhs1")
    rhs2 = big.tile([K2, H], BF16, tag="rhs2", name="rhs2")
    v_sb = big.tile([MB, NMB, 4], BF16, tag="v", name="v")
    nc.sync.dma_start(out=lhs1, in_=names["lhs1"])
    nc.sync.dma_start(out=lhs2, in_=names["lhs2"])
    nc.sync.dma_start(out=rhs1, in_=names["rhs1"])
    nc.sync.dma_start(out=rhs2, in_=names["rhs2"])
    nc.sync.dma_start(out=v_sb, in_=names["v"].rearrange("p (c f) -> p c f", f=4))

    w = [
        wpool.tile([4, NB], F32, tag=f"w{j}", name=f"w{j}") for j in range(NNB)
    ]
    for mi in range(NMB):
        mc = slice(mi * MB, (mi + 1) * MB)
        s = [
            spool.tile([MB, WACT], F32, tag="s", name=f"s{k}") for k in range(2)
        ]
        # scores: batch matmuls by weight set so LDWEIGHTS stays warm
        for k in range(2):
            for half in range(2):
                nj = 2 * k + half
                nc.tensor.matmul(
                    s[k][:, half * NB : (half + 1) * NB],
                    lhsT=lhs1[:, mc],
                    rhs=rhs1[:, nj * NB : (nj + 1) * NB],
                    start=True,
                    stop=False,
                )
        for k in range(2):
            for half in range(2):
                nj = 2 * k + half
                nc.tensor.matmul(
                    s[k][:, half * NB : (half + 1) * NB],
                    lhsT=lhs2[:, mc],
                    rhs=rhs2[:, nj * NB : (nj + 1) * NB],
                    start=False,
                    stop=True,
                )
        u = []
        for k in range(2):
            u_t = upool.tile([MB, WACT], BF16, tag="u", name=f"u{k}")
            nc.scalar.activation(
                out=u_t, in_=s[k], func=mybir.ActivationFunctionType.Exp, scale=2.0
            )
            u.append(u_t)
        for k in range(2):
            for half in range(2):
                nj = 2 * k + half
                nc.tensor.matmul(
                    w[nj],
                    lhsT=v_sb[:, mi, :],
                    rhs=u[k][:, half * NB : (half + 1) * NB],
                    start=(mi == 0),
                    stop=(mi == NMB - 1),
                )

    for nj in range(NNB):
        w_sb = epool.tile([4, NB], F32, tag="wsb", name="wsb")
        nc.scalar.copy(w_sb, w[nj])
        for j2 in range(NB // MB):
            wt_ps = spool.tile([MB, 4], F32, tag="s", name="wt")
            nc.tensor.transpose(
                wt_ps, w_sb[:, j2 * MB : (j2 + 1) * MB], identity[:4, :4]
            )
            wt_sb = epool.tile([MB, 4], F32, tag="wtsb", name="wtsb")
            nc.vector.tensor_copy(wt_sb, wt_ps)
            r_sb = epool.tile([MB, 1], F32, tag="r", name="r")
            nc.vector.reciprocal(r_sb, wt_sb[:, 3:4])
            o_sb = epool.tile([MB, 3], F32, tag="o", name="o")
            nc.vector.tensor_scalar_mul(o_sb, wt_sb[:, 0:3], r_sb)
            row = nj * NB + j2 * MB
            nc.sync.dma_start(out=out_d[row : row + MB, :], in_=o_sb)


def _build():
    nc = bacc.Bacc("TRN2", target_bir_lowering=False, debug=False)

    def din(name, shape):
        return nc.dram_tensor(name, shape, BF16, kind="ExternalInput").ap()

    sides = []
    for side in ("A", "B"):
        sides.append(
            {
                "lhs1": din(f"lhs1{side}", [2 * D, M]),
                "lhs2": din(f"lhs2{side}", [K2, M]),
                "rhs1": din(f"rhs1{side}", [2 * D, H]),
                "rhs2": din(f"rhs2{side}", [K2, H]),
                "v": din(f"v{side}", [MB, NMB * 4]),
            }
        )
    out_src = nc.dram_tensor("out_src", [H, 3], F32, kind="ExternalOutput").ap()
    out_tgt = nc.dram_tensor("out_tgt", [H, 3], F32, kind="ExternalOutput").ap()

    with tile.TileContext(nc) as tc:
        with (
            tc.tile_pool(name="big", bufs=2) as big,
            tc.tile_pool(name="upool", bufs=4) as upool,
            tc.tile_pool(name="spool", bufs=2, space="PSUM") as spool,
            tc.tile_pool(name="wpool", bufs=1, space="PSUM") as wpool,
            tc.tile_pool(name="epool", bufs=2) as epool,
            tc.tile_pool(name="ident", bufs=1) as ident,
        ):
            identity = ident.tile([MB, MB], F32, tag="identity", name="identity")
            make_identity(nc, identity[:])
            pools = (big, upool, spool, wpool, epool)
            _build_side(nc, pools, identity, sides[0], out_src)
            _build_side(nc, pools, identity, sides[1], out_tgt)

    nc.compile()
    return nc


def _hi_lo(x):
    hi = x.astype(NPBF)
    lo = (x - hi.astype(np.float32)).astype(NPBF)
    return hi, lo


def _prep_inputs(src, tgt, src_f, tgt_f):
    """Build the 8 per-core input maps (host-side sharding + layout prep)."""
    src = np.ascontiguousarray(src, dtype=np.float32)
    tgt = np.ascontiguousarray(tgt, dtype=np.float32)
    src_f = np.ascontiguousarray(src_f, dtype=np.float32)
    tgt_f = np.ascontiguousarray(tgt_f, dtype=np.float32)
    aa = (src_f * src_f).sum(axis=1)  # [B, N]
    bb = (tgt_f * tgt_f).sum(axis=1)  # [B, M]

    def chunk_v(pts):  # [L, 3] -> [MB, (L//MB)*4] bf16
        v = np.concatenate([pts, np.ones((pts.shape[0], 1), np.float32)], axis=1)
        return np.ascontiguousarray(
            v.reshape(-1, MB, 4).transpose(1, 0, 2).reshape(MB, -1).astype(NPBF)
        )

    def side(L, R, bias_m, shift_n, vpts, sl):
        ones_m = np.ones((1, L.shape[1]), NPBF)
        ones_n = np.ones((1, H), NPBF)
        Lhi, Llo = _hi_lo(L)
        Rhi, Rlo = _hi_lo(R[:, sl])
        bhi, blo = _hi_lo((C0 - bias_m) * 0.5)
        shift = (-0.5 * shift_n[sl]).astype(NPBF)
        return {
            "lhs1": np.ascontiguousarray(np.vstack([Lhi, Llo])),
            "lhs2": np.ascontiguousarray(
                np.vstack([Lhi, ones_m, bhi[None, :], blo[None, :]])
            ),
            "rhs1": np.ascontiguousarray(np.vstack([Rhi, Rhi])),
            "rhs2": np.ascontiguousarray(
                np.vstack([Rlo, shift[None, :], ones_n, ones_n])
            ),
            "v": chunk_v(vpts),
        }

    in_maps = []
    for c in range(NCORES):
        b, h = divmod(c, 2)
        sl = slice(h * H, (h + 1) * H)
        A = side(tgt_f[b], src_f[b], bb[b], aa[b], tgt[b], sl)
        Bs = side(src_f[b], tgt_f[b], aa[b], bb[b], src[b], sl)
        m = {k + "A": v for k, v in A.items()}
        m.update({k + "B": v for k, v in Bs.items()})
        in_maps.append(m)
    return in_maps


def run(inputs, trace=False, **kw):
    global _PROG
    if _PROG is None:
        _PROG = _build()
    in_maps = _prep_inputs(
        inputs["src"], inputs["tgt"], inputs["src_f"], inputs["tgt_f"]
    )
    bkr = run_bass_kernel_spmd(
        _PROG, in_maps, core_ids=list(range(NCORES)), trace=trace, **kw
    )
    src_corr = np.zeros((B, N, 3), np.float32)
    tgt_corr = np.zeros((B, M, 3), np.float32)
    for c in range(NCORES):
        b, h = divmod(c, 2)
        sl = slice(h * H, (h + 1) * H)
        src_corr[b, sl] = bkr.results[c]["out_src"]
        tgt_corr[b, sl] = bkr.results[c]["out_tgt"]
    return (src_corr, tgt_corr), bkr


def kernel(**inputs):
    out, _ = run(inputs)
    return out


# revision 4
# speedup vs baseline: 2.9963x; 1.6674x over previous
"""Trainium2 Bass kernel for nn_ComputeCorr (retrieval_knn).

Math (per batch pair b, D=64 features):
  d[n,m] = ||sf[n]-tf[m]||^2,  sf = src_f[b].T, tf = tgt_f[b].T
  src_corr[b] = softmax_m(-d) @ tgt[b];  tgt_corr[b] = softmax_n(-d.T) @ src[b]

Restructure (per side, shown for src_corr):
  softmax_m(-d)[n,:] @ tgt = (sum_m U[m,n] * [tgt|1][m,:])[:3] / (...)[3]
  U[m,n] = exp(c0 - d[n,m]) computed directly in [m(part), n(free)] layout.
  The whole exponent (2*ab - aa[n] - bb[m] + c0)/2 comes from ONE fp16
  matmul with K-packed augmentation rows (K = 64 + 3):
    lhsT = [fp16(L); 1; bias_hi; bias_lo]   (bias = (c0 - |l_m|^2)/2)
    rhs  = [fp16(R); -|r_n|^2/2; 1; 1]
  so exponent == c0 - d <= c0: no max pass, no overflow, and the bf16/fp16
  rounding of the -|r_n|^2/2 row is a per-column shift that cancels in the
  softmax normalization.  ScalarE does a bias-free exp (scale=2.0) from
  PSUM over 1024-wide tiles, emitting U in bf16; one bf16 matmul against
  [tgt|1] accumulates numerator+denominator in PSUM over all 32 m-chunks
  (the stationary [tgt|1] slice is widened to 128 columns so FWL kicks in;
  output rows 4..127 are garbage we never read).  Epilogue: tiny PE
  transpose to [n, 4], reciprocal + multiply on DVE, DMA out.

Sharding: 8 cores = 4 batches x 2 halves; core c = batch c//2, rows
[h*2048,(h+1)*2048) of BOTH outputs (h=c%2). The [4096 x 2048] score
block per side is never materialized in DRAM.
"""

import os
import sys

import numpy as np

for _p in ("/opt/trn_rl_repo", "/root/.axon_site/_ro/trn_rl_repo"):
    if os.path.isdir(_p) and _p not in sys.path:
        sys.path.insert(0, _p)

import ml_dtypes

import concourse.bacc as bacc
import concourse.tile as tile
from concourse import mybir
from concourse.bass_utils import run_bass_kernel_spmd
from concourse.masks import make_identity

B, N, M, D = 4, 4096, 4096, 64
H = N // 2  # rows per core per side
NCORES = 8
C0 = 40.0
KS = D + 3  # score matmul contraction: features + shift row + 2 bias rows
MB = 128  # m block (score partition dim)
NB = 512  # matmul free dim (PSUM bank)
NMB = M // MB  # 32 m blocks
NNB = H // NB  # 4 n blocks per core
VW = 256  # padded width of the v tensor (128-wide lhsT slices)
F32 = mybir.dt.float32
F16 = mybir.dt.float16
BF16 = mybir.dt.bfloat16
NPBF = ml_dtypes.bfloat16

_PROG = None


def _build_side(nc, pools, identity, names, out_d):
    big, upool, spool, wpool, epool = pools

    lhs = big.tile([KS, M], F16, tag="lhs", name="lhs")
    rhs = big.tile([KS, H], F16, tag="rhs", name="rhs")
    v_sb = big.tile([MB, VW], BF16, tag="v", name="v")
    nc.sync.dma_start(out=lhs, in_=names["lhs"])
    nc.sync.dma_start(out=rhs, in_=names["rhs"])
    nc.sync.dma_start(out=v_sb, in_=names["v"])

    for nj in range(NNB):
        w = wpool.tile([MB, NB], F32, tag="w", name="w")
        ncol = slice(nj * NB, (nj + 1) * NB)
        for mp in range(NMB // 2):  # mi pairs
            s = spool.tile([MB, 2 * NB], F32, tag="s", name="s")
            u = upool.tile([MB, 2 * NB], BF16, tag="u", name="u")
            for half in range(2):
                mi = 2 * mp + half
                nc.tensor.matmul(
                    s[:, half * NB : (half + 1) * NB],
                    lhsT=lhs[:, mi * MB : (mi + 1) * MB],
                    rhs=rhs[:, ncol],
                    start=True,
                    stop=True,
                )
            nc.scalar.activation(
                out=u, in_=s, func=mybir.ActivationFunctionType.Exp, scale=2.0
            )
            for half in range(2):
                mi = 2 * mp + half
                nc.tensor.matmul(
                    w,
                    lhsT=v_sb[:, mi * 4 : mi * 4 + MB],
                    rhs=u[:, half * NB : (half + 1) * NB],
                    start=(mi == 0),
                    stop=(mi == NMB - 1),
                )
        # epilogue for this n block: W[0:4,:] = [num_xyz; denom] -> out rows
        w_sb = epool.tile([4, NB], F32, tag="wsb", name="wsb")
        nc.scalar.copy(w_sb, w[0:4, :])
        for j2 in range(NB // MB):
            wt_ps = spool.tile([MB, 4], F32, tag="s", name="wt")
            nc.tensor.transpose(
                wt_ps, w_sb[:, j2 * MB : (j2 + 1) * MB], identity[:4, :4]
            )
            wt_sb = epool.tile([MB, 4], F32, tag="wtsb", name="wtsb")
            nc.vector.tensor_copy(wt_sb, wt_ps)
            r_sb = epool.tile([MB, 1], F32, tag="r", name="r")
            nc.vector.reciprocal(r_sb, wt_sb[:, 3:4])
            o_sb = epool.tile([MB, 3], F32, tag="o", name="o")
            nc.vector.tensor_scalar_mul(o_sb, wt_sb[:, 0:3], r_sb)
            row = nj * NB + j2 * MB
            nc.sync.dma_start(out=out_d[row : row + MB, :], in_=o_sb)


def _build():
    nc = bacc.Bacc("TRN2", target_bir_lowering=False, debug=False)

    sides = []
    for side in ("A", "B"):
        sides.append(
            {
                "lhs": nc.dram_tensor(
                    f"lhs{side}", [KS, M], F16, kind="ExternalInput"
                ).ap(),
                "rhs": nc.dram_tensor(
                    f"rhs{side}", [KS, H], F16, kind="ExternalInput"
                ).ap(),
                "v": nc.dram_tensor(
                    f"v{side}", [MB, VW], BF16, kind="ExternalInput"
                ).ap(),
            }
        )
    out_src = nc.dram_tensor("out_src", [H, 3], F32, kind="ExternalOutput").ap()
    out_tgt = nc.dram_tensor("out_tgt", [H, 3], F32, kind="ExternalOutput").ap()

    with tile.TileContext(nc) as tc:
        with (
            tc.tile_pool(name="big", bufs=2) as big,
            tc.tile_pool(name="upool", bufs=4) as upool,
            tc.tile_pool(name="spool", bufs=3, space="PSUM") as spool,
            tc.tile_pool(name="wpool", bufs=2, space="PSUM") as wpool,
            tc.tile_pool(name="epool", bufs=2) as epool,
            tc.tile_pool(name="ident", bufs=1) as ident,
        ):
            identity = ident.tile([MB, MB], F32, tag="identity", name="identity")
            make_identity(nc, identity[:])
            pools = (big, upool, spool, wpool, epool)
            _build_side(nc, pools, identity, sides[0], out_src)
            _build_side(nc, pools, identity, sides[1], out_tgt)

    nc.compile()
    return nc


def _hi_lo16(x):
    hi = x.astype(np.float16)
    lo = (x - hi.astype(np.float32)).astype(np.float16)
    return hi, lo


def _prep_inputs(src, tgt, src_f, tgt_f):
    """Build the 8 per-core input maps (host-side sharding + layout prep)."""
    src = np.ascontiguousarray(src, dtype=np.float32)
    tgt = np.ascontiguousarray(tgt, dtype=np.float32)
    src_f = np.ascontiguousarray(src_f, dtype=np.float32)
    tgt_f = np.ascontiguousarray(tgt_f, dtype=np.float32)
    aa = (src_f * src_f).sum(axis=1)  # [B, N]
    bb = (tgt_f * tgt_f).sum(axis=1)  # [B, M]

    def chunk_v(pts):  # [L, 3] -> [MB, VW] bf16, col 4*c+f = [pts|1][c*MB+p, f]
        v = np.concatenate([pts, np.ones((pts.shape[0], 1), np.float32)], axis=1)
        flat = v.reshape(-1, MB, 4).transpose(1, 0, 2).reshape(MB, -1)
        out = np.zeros((MB, VW), np.float32)
        out[:, : flat.shape[1]] = flat
        return np.ascontiguousarray(out.astype(NPBF))

    def side(L, R, bias_m, shift_n, vpts, sl):
        ones_m = np.ones((1, L.shape[1]), np.float16)
        ones_n = np.ones((1, H), np.float16)
        bh, bl = _hi_lo16((C0 - bias_m) * 0.5)
        shift = (-0.5 * shift_n[sl]).astype(np.float16)
        return {
            "lhs": np.ascontiguousarray(
                np.vstack([L.astype(np.float16), ones_m, bh[None, :], bl[None, :]])
            ),
            "rhs": np.ascontiguousarray(
                np.vstack([R[:, sl].astype(np.float16), shift[None, :], ones_n, ones_n])
            ),
            "v": chunk_v(vpts),
        }

    in_maps = []
    for c in range(NCORES):
        b, h = divmod(c, 2)
        sl = slice(h * H, (h + 1) * H)
        A = side(tgt_f[b], src_f[b], bb[b], aa[b], tgt[b], sl)
        Bs = side(src_f[b], tgt_f[b], aa[b], bb[b], src[b], sl)
        m = {k + "A": v for k, v in A.items()}
        m.update({k + "B": v for k, v in Bs.items()})
        in_maps.append(m)
    return in_maps


def run(inputs, trace=False, **kw):
    global _PROG
    if _PROG is None:
        _PROG = _build()
    in_maps = _prep_inputs(
        inputs["src"], inputs["tgt"], inputs["src_f"], inputs["tgt_f"]
    )
    bkr = run_bass_kernel_spmd(
        _PROG, in_maps, core_ids=list(range(NCORES)), trace=trace, **kw
    )
    src_corr = np.zeros((B, N, 3), np.float32)
    tgt_corr = np.zeros((B, M, 3), np.float32)
    for c in range(NCORES):
        b, h = divmod(c, 2)
        sl = slice(h * H, (h + 1) * H)
        src_corr[b, sl] = bkr.results[c]["out_src"]
        tgt_corr[b, sl] = bkr.results[c]["out_tgt"]
    return (src_corr, tgt_corr), bkr


def kernel(**inputs):
    out, _ = run(inputs)
    return out


# revision 8
# speedup vs baseline: 3.0230x; 1.0089x over previous
"""Trainium2 Bass kernel for nn_ComputeCorr (retrieval_knn).

Math (per batch pair b, D=64 features):
  d[n,m] = ||sf[n]-tf[m]||^2,  sf = src_f[b].T, tf = tgt_f[b].T
  src_corr[b] = softmax_m(-d) @ tgt[b];  tgt_corr[b] = softmax_n(-d.T) @ src[b]

Restructure (per side, shown for src_corr):
  softmax_m(-d)[n,:] @ tgt = (sum_m U[m,n] * [tgt|1][m,:])[:3] / (...)[3]
  U[m,n] = exp(c0 - d[n,m]) computed directly in [m(part), n(free)] layout.
  The whole exponent (2*ab - aa[n] - bb[m] + c0)/2 comes from ONE fp16
  matmul with K-packed augmentation rows (K = 64 + 3):
    lhsT = [fp16(L); 1; bias_hi; bias_lo]   (bias = (c0 - |l_m|^2)/2)
    rhs  = [fp16(R); -|r_n|^2/2; 1; 1]
  so exponent == c0 - d <= c0: no max pass, no overflow, and the bf16/fp16
  rounding of the -|r_n|^2/2 row is a per-column shift that cancels in the
  softmax normalization.  ScalarE does a bias-free exp (scale=2.0) from
  PSUM over 1024-wide tiles, emitting U in bf16; one bf16 matmul against
  [tgt|1] accumulates numerator+denominator in PSUM over all 32 m-chunks
  (the stationary [tgt|1] slice is widened to 128 columns so FWL kicks in;
  output rows 4..127 are garbage we never read).  Epilogue: tiny PE
  transpose to [n, 4], reciprocal + multiply on DVE, DMA out.

Sharding: 8 cores = 4 batches x 2 halves; core c = batch c//2, rows
[h*2048,(h+1)*2048) of BOTH outputs (h=c%2). The [4096 x 2048] score
block per side is never materialized in DRAM.
"""

import os
import sys

import numpy as np

for _p in ("/opt/trn_rl_repo", "/root/.axon_site/_ro/trn_rl_repo"):
    if os.path.isdir(_p) and _p not in sys.path:
        sys.path.insert(0, _p)

import ml_dtypes

import concourse.bacc as bacc
import concourse.tile as tile
from concourse import mybir
from concourse.bass_utils import run_bass_kernel_spmd
from concourse.masks import make_identity

B, N, M, D = 4, 4096, 4096, 64
H = N // 2  # rows per core per side
NCORES = 8
C0 = 40.0
KS = D + 3  # score matmul contraction: features + shift row + 2 bias rows
MB = 128  # m block (score partition dim)
NB = 512  # matmul free dim (PSUM bank)
NMB = M // MB  # 32 m blocks
NNB = H // NB  # 4 n blocks per core
VW = 256  # padded width of the v tensor (128-wide lhsT slices)
F32 = mybir.dt.float32
F16 = mybir.dt.float16
BF16 = mybir.dt.bfloat16
NPBF = ml_dtypes.bfloat16

_PROG = None


LCH = 4  # lhs DMA column chunks
LCW = M // LCH  # 1024 columns per chunk


def _build_side(nc, pools, identity, names, out_d):
    big, upool, spool, wpool, epool = pools

    # Interleave input DMAs over both HWDGE rings (sync + scalar) and chunk
    # lhs by columns so mi=0 compute starts as soon as the first chunk lands.
    rings = [nc.sync, nc.scalar] if os.environ.get("TWO_RINGS") else [nc.sync, nc.sync]
    lhs_ch = [
        big.tile([KS, LCW], F16, tag=f"lhs{c}", name=f"lhs{c}") for c in range(LCH)
    ]
    for c in range(LCH):
        rings[c % 2].dma_start(
            out=lhs_ch[c], in_=names["lhs"][:, c * LCW : (c + 1) * LCW]
        )
    v_sb = big.tile([MB, VW], F16, tag="v", name="v")
    rings[0].dma_start(out=v_sb, in_=names["v"])
    rhs = big.tile([KS, H], F16, tag="rhs", name="rhs")
    for c in range(2):
        rings[(c + 1) % 2].dma_start(
            out=rhs[:, c * (H // 2) : (c + 1) * (H // 2)],
            in_=names["rhs"][:, c * (H // 2) : (c + 1) * (H // 2)],
        )

    def lhs_slice(mi):
        c, o = divmod(mi * MB, LCW)
        return lhs_ch[c][:, o : o + MB]

    for nj in range(NNB):
        w = wpool.tile([MB, NB], F32, tag="w", name="w")
        ncol = slice(nj * NB, (nj + 1) * NB)
        for mp in range(NMB // 2):  # mi pairs
            s = spool.tile([MB, 2 * NB], F32, tag="s", name="s")
            u = upool.tile([MB, 2 * NB], BF16, tag="u", name="u")
            for half in range(2):
                mi = 2 * mp + half
                nc.tensor.matmul(
                    s[:, half * NB : (half + 1) * NB],
                    lhsT=lhs_slice(mi),
                    rhs=rhs[:, ncol],
                    start=True,
                    stop=True,
                )
            nc.scalar.activation(
                out=u, in_=s, func=mybir.ActivationFunctionType.Exp, scale=2.0
            )
            for half in range(2):
                mi = 2 * mp + half
                nc.tensor.matmul(
                    w,
                    lhsT=v_sb[:, mi * 4 : mi * 4 + MB],
                    rhs=u[:, half * NB : (half + 1) * NB],
                    start=(mi == 0),
                    stop=(mi == NMB - 1),
                )
        # epilogue for this n block: W[0:4,:] = [num_xyz; denom] -> out rows
        w_sb = epool.tile([4, NB], F32, tag="wsb", name="wsb")
        nc.scalar.copy(w_sb, w[0:4, :])
        for j2 in range(NB // MB):
            wt_ps = spool.tile([MB, 4], F32, tag="s", name="wt")
            nc.tensor.transpose(
                wt_ps, w_sb[:, j2 * MB : (j2 + 1) * MB], identity[:4, :4]
            )
            wt_sb = epool.tile([MB, 4], F32, tag="wtsb", name="wtsb")
            nc.vector.tensor_copy(wt_sb, wt_ps)
            r_sb = epool.tile([MB, 1], F32, tag="r", name="r")
            nc.vector.reciprocal(r_sb, wt_sb[:, 3:4])
            o_sb = epool.tile([MB, 3], F32, tag="o", name="o")
            nc.vector.tensor_scalar_mul(o_sb, wt_sb[:, 0:3], r_sb)
            row = nj * NB + j2 * MB
            nc.sync.dma_start(out=out_d[row : row + MB, :], in_=o_sb)


def _build():
    nc = bacc.Bacc("TRN2", target_bir_lowering=False, debug=False)

    sides = []
    for side in ("A", "B"):
        sides.append(
            {
                "lhs": nc.dram_tensor(
                    f"lhs{side}", [KS, M], F16, kind="ExternalInput"
                ).ap(),
                "rhs": nc.dram_tensor(
                    f"rhs{side}", [KS, H], F16, kind="ExternalInput"
                ).ap(),
                "v": nc.dram_tensor(
                    f"v{side}", [MB, VW], F16, kind="ExternalInput"
                ).ap(),
            }
        )
    out_src = nc.dram_tensor("out_src", [H, 3], F32, kind="ExternalOutput").ap()
    out_tgt = nc.dram_tensor("out_tgt", [H, 3], F32, kind="ExternalOutput").ap()

    with tile.TileContext(nc) as tc:
        with (
            tc.tile_pool(name="big", bufs=2) as big,
            tc.tile_pool(name="upool", bufs=4) as upool,
            tc.tile_pool(name="spool", bufs=3, space="PSUM") as spool,
            tc.tile_pool(name="wpool", bufs=2, space="PSUM") as wpool,
            tc.tile_pool(name="epool", bufs=2) as epool,
            tc.tile_pool(name="ident", bufs=1) as ident,
        ):
            identity = ident.tile([MB, MB], F32, tag="identity", name="identity")
            make_identity(nc, identity[:])
            pools = (big, upool, spool, wpool, epool)
            _build_side(nc, pools, identity, sides[0], out_src)
            _build_side(nc, pools, identity, sides[1], out_tgt)

    nc.compile()
    return nc


def _hi_lo16(x):
    hi = x.astype(np.float16)
    lo = (x - hi.astype(np.float32)).astype(np.float16)
    return hi, lo


def _prep_inputs(src, tgt, src_f, tgt_f):
    """Build the 8 per-core input maps (host-side sharding + layout prep)."""
    src = np.ascontiguousarray(src, dtype=np.float32)
    tgt = np.ascontiguousarray(tgt, dtype=np.float32)
    src_f = np.ascontiguousarray(src_f, dtype=np.float32)
    tgt_f = np.ascontiguousarray(tgt_f, dtype=np.float32)
    aa = (src_f * src_f).sum(axis=1)  # [B, N]
    bb = (tgt_f * tgt_f).sum(axis=1)  # [B, M]

    def chunk_v(pts):  # [L, 3] -> [MB, VW] bf16, col 4*c+f = [pts|1][c*MB+p, f]
        v = np.concatenate([pts, np.ones((pts.shape[0], 1), np.float32)], axis=1)
        flat = v.reshape(-1, MB, 4).transpose(1, 0, 2).reshape(MB, -1)
        out = np.zeros((MB, VW), np.float32)
        out[:, : flat.shape[1]] = flat
        return np.ascontiguousarray(out.astype(np.float16))

    def side(L, R, bias_m, shift_n, vpts, sl):
        ones_m = np.ones((1, L.shape[1]), np.float16)
        ones_n = np.ones((1, H), np.float16)
        bh, bl = _hi_lo16((C0 - bias_m) * 0.5)
        shift = (-0.5 * shift_n[sl]).astype(np.float16)
        return {
            "lhs": np.ascontiguousarray(
                np.vstack([L.astype(np.float16), ones_m, bh[None, :], bl[None, :]])
            ),
            "rhs": np.ascontiguousarray(
                np.vstack([R[:, sl].astype(np.float16), shift[None, :], ones_n, ones_n])
            ),
            "v": chunk_v(vpts),
        }

    in_maps = []
    for c in range(NCORES):
        b, h = divmod(c, 2)
        sl = slice(h * H, (h + 1) * H)
        A = side(tgt_f[b], src_f[b], bb[b], aa[b], tgt[b], sl)
        Bs = side(src_f[b], tgt_f[b], aa[b], bb[b], src[b], sl)
        m = {k + "A": v for k, v in A.items()}
        m.update({k + "B": v for k, v in Bs.items()})
        in_maps.append(m)
    return in_maps


def run(inputs, trace=False, **kw):
    global _PROG
    if _PROG is None:
        _PROG = _build()
    in_maps = _prep_inputs(
        inputs["src"], inputs["tgt"], inputs["src_f"], inputs["tgt_f"]
    )
    bkr = run_bass_kernel_spmd(
        _PROG, in_maps, core_ids=list(range(NCORES)), trace=trace, **kw
    )
    src_corr = np.zeros((B, N, 3), np.float32)
    tgt_corr = np.zeros((B, M, 3), np.float32)
    for c in range(NCORES):
        b, h = divmod(c, 2)
        sl = slice(h * H, (h + 1) * H)
        src_corr[b, sl] = bkr.results[c]["out_src"]
        tgt_corr[b, sl] = bkr.results[c]["out_tgt"]
    return (src_corr, tgt_corr), bkr


def kernel(**inputs):
    out, _ = run(inputs)
    return out


# revision 10
# speedup vs baseline: 3.0427x; 1.0065x over previous
"""Trainium2 Bass kernel for nn_ComputeCorr (retrieval_knn).

Math (per batch pair b, D=64 features):
  d[n,m] = ||sf[n]-tf[m]||^2,  sf = src_f[b].T, tf = tgt_f[b].T
  src_corr[b] = softmax_m(-d) @ tgt[b];  tgt_corr[b] = softmax_n(-d.T) @ src[b]

Restructure (per side, shown for src_corr):
  softmax_m(-d)[n,:] @ tgt = (sum_m U[m,n] * [tgt|1][m,:])[:3] / (...)[3]
  U[m,n] = exp(c0 - d[n,m]) computed directly in [m(part), n(free)] layout.
  The whole exponent (2*ab - aa[n] - bb[m] + c0)/2 comes from ONE fp16
  matmul with K-packed augmentation rows (K = 64 + 3):
    lhsT = [fp16(L); 1; bias_hi; bias_lo]   (bias = (c0 - |l_m|^2)/2)
    rhs  = [fp16(R); -|r_n|^2/2; 1; 1]
  so exponent == c0 - d <= c0: no max pass, no overflow, and the bf16/fp16
  rounding of the -|r_n|^2/2 row is a per-column shift that cancels in the
  softmax normalization.  ScalarE does a bias-free exp (scale=2.0) from
  PSUM over 1024-wide tiles, emitting U in bf16; one bf16 matmul against
  [tgt|1] accumulates numerator+denominator in PSUM over all 32 m-chunks
  (the stationary [tgt|1] slice is widened to 128 columns so FWL kicks in;
  output rows 4..127 are garbage we never read).  Epilogue: tiny PE
  transpose to [n, 4], reciprocal + multiply on DVE, DMA out.

Sharding: 8 cores = 4 batches x 2 halves; core c = batch c//2, rows
[h*2048,(h+1)*2048) of BOTH outputs (h=c%2). The [4096 x 2048] score
block per side is never materialized in DRAM.
"""

import os
import sys

import numpy as np

for _p in ("/opt/trn_rl_repo", "/root/.axon_site/_ro/trn_rl_repo"):
    if os.path.isdir(_p) and _p not in sys.path:
        sys.path.insert(0, _p)

import ml_dtypes

import concourse.bacc as bacc
import concourse.tile as tile
from concourse import mybir
from concourse.bass_utils import run_bass_kernel_spmd
from concourse.masks import make_identity

B, N, M, D = 4, 4096, 4096, 64
H = N // 2  # rows per core per side
NCORES = 8
C0 = 40.0
KS = D + 3  # score matmul contraction: features + shift row + 2 bias rows
MB = 128  # m block (score partition dim)
NB = 512  # matmul free dim (PSUM bank)
NMB = M // MB  # 32 m blocks
NNB = H // NB  # 4 n blocks per core
VW = 256  # padded width of the v tensor (128-wide lhsT slices)
F32 = mybir.dt.float32
F16 = mybir.dt.float16
BF16 = mybir.dt.bfloat16
NPBF = ml_dtypes.bfloat16

_PROG = None


LCH = 4  # lhs DMA column chunks
LCW = M // LCH  # 1024 columns per chunk


def _build_side(nc, pools, identity, names, out_d, ring):
    big, upool, spool, wpool, epool = pools
    side = names["side"]

    # One DMA ring per side (sync HWDGE for A, gpsimd SWDGE for B) — in this
    # runtime each ring drains through a single SDMA engine at ~23GB/s, so
    # the two sides' inputs must ride different rings to overlap.  Issue in
    # compute-consumption order: rhs half 0 and lhs chunk 0 gate the first
    # matmul; the rest streams in behind the compute.
    lhs_ch = [
        big.tile([KS, LCW], F16, tag=f"lhs{c}{side}", name=f"lhs{c}")
        for c in range(LCH)
    ]
    rhs = big.tile([KS, H], F16, tag=f"rhs{side}", name="rhs")
    v_sb = big.tile([MB, VW], F16, tag=f"v{side}", name="v")
    ring.dma_start(out=rhs[:, : H // 2], in_=names["rhs"][:, : H // 2])
    ring.dma_start(out=lhs_ch[0], in_=names["lhs"][:, :LCW])
    ring.dma_start(out=v_sb, in_=names["v"])
    ring.dma_start(out=lhs_ch[1], in_=names["lhs"][:, LCW : 2 * LCW])
    ring.dma_start(out=rhs[:, H // 2 :], in_=names["rhs"][:, H // 2 :])
    ring.dma_start(out=lhs_ch[2], in_=names["lhs"][:, 2 * LCW : 3 * LCW])
    ring.dma_start(out=lhs_ch[3], in_=names["lhs"][:, 3 * LCW :])

    def lhs_slice(mi):
        c, o = divmod(mi * MB, LCW)
        return lhs_ch[c][:, o : o + MB]

    for nj in range(NNB):
        w = wpool.tile([MB, NB], F32, tag="w", name="w")
        ncol = slice(nj * NB, (nj + 1) * NB)
        for mp in range(NMB // 2):  # mi pairs
            s = spool.tile([MB, 2 * NB], F32, tag="s", name="s")
            u = upool.tile([MB, 2 * NB], BF16, tag="u", name="u")
            for half in range(2):
                mi = 2 * mp + half
                nc.tensor.matmul(
                    s[:, half * NB : (half + 1) * NB],
                    lhsT=lhs_slice(mi),
                    rhs=rhs[:, ncol],
                    start=True,
                    stop=True,
                )
            nc.scalar.activation(
                out=u, in_=s, func=mybir.ActivationFunctionType.Exp, scale=2.0
            )
            for half in range(2):
                mi = 2 * mp + half
                nc.tensor.matmul(
                    w,
                    lhsT=v_sb[:, mi * 4 : mi * 4 + MB],
                    rhs=u[:, half * NB : (half + 1) * NB],
                    start=(mi == 0),
                    stop=(mi == NMB - 1),
                )
        # epilogue for this n block: W[0:4,:] = [num_xyz; denom] -> out rows
        w_sb = epool.tile([4, NB], F32, tag="wsb", name="wsb")
        nc.scalar.copy(w_sb, w[0:4, :])
        for j2 in range(NB // MB):
            wt_ps = spool.tile([MB, 4], F32, tag="s", name="wt")
            nc.tensor.transpose(
                wt_ps, w_sb[:, j2 * MB : (j2 + 1) * MB], identity[:4, :4]
            )
            wt_sb = epool.tile([MB, 4], F32, tag="wtsb", name="wtsb")
            nc.vector.tensor_copy(wt_sb, wt_ps)
            r_sb = epool.tile([MB, 1], F32, tag="r", name="r")
            nc.vector.reciprocal(r_sb, wt_sb[:, 3:4])
            o_sb = epool.tile([MB, 3], F32, tag="o", name="o")
            nc.vector.tensor_scalar_mul(o_sb, wt_sb[:, 0:3], r_sb)
            row = nj * NB + j2 * MB
            ring.dma_start(out=out_d[row : row + MB, :], in_=o_sb)


def _build():
    nc = bacc.Bacc("TRN2", target_bir_lowering=False, debug=False)

    sides = []
    for side in ("A", "B"):
        sides.append(
            {
                "side": side,
                "lhs": nc.dram_tensor(
                    f"lhs{side}", [KS, M], F16, kind="ExternalInput"
                ).ap(),
                "rhs": nc.dram_tensor(
                    f"rhs{side}", [KS, H], F16, kind="ExternalInput"
                ).ap(),
                "v": nc.dram_tensor(
                    f"v{side}", [MB, VW], F16, kind="ExternalInput"
                ).ap(),
            }
        )
    out_src = nc.dram_tensor("out_src", [H, 3], F32, kind="ExternalOutput").ap()
    out_tgt = nc.dram_tensor("out_tgt", [H, 3], F32, kind="ExternalOutput").ap()

    with tile.TileContext(nc) as tc:
        with (
            tc.tile_pool(name="big", bufs=2) as big,
            tc.tile_pool(name="upool", bufs=4) as upool,
            tc.tile_pool(name="spool", bufs=3, space="PSUM") as spool,
            tc.tile_pool(name="wpool", bufs=2, space="PSUM") as wpool,
            tc.tile_pool(name="epool", bufs=2) as epool,
            tc.tile_pool(name="ident", bufs=1) as ident,
        ):
            identity = ident.tile([MB, MB], F32, tag="identity", name="identity")
            make_identity(nc, identity[:])
            pools = (big, upool, spool, wpool, epool)
            _build_side(nc, pools, identity, sides[0], out_src, nc.sync)
            _build_side(nc, pools, identity, sides[1], out_tgt, nc.gpsimd)

    nc.compile()
    return nc


def _hi_lo16(x):
    hi = x.astype(np.float16)
    lo = (x - hi.astype(np.float32)).astype(np.float16)
    return hi, lo


def _prep_inputs(src, tgt, src_f, tgt_f):
    """Build the 8 per-core input maps (host-side sharding + layout prep)."""
    src = np.ascontiguousarray(src, dtype=np.float32)
    tgt = np.ascontiguousarray(tgt, dtype=np.float32)
    src_f = np.ascontiguousarray(src_f, dtype=np.float32)
    tgt_f = np.ascontiguousarray(tgt_f, dtype=np.float32)
    aa = (src_f * src_f).sum(axis=1)  # [B, N]
    bb = (tgt_f * tgt_f).sum(axis=1)  # [B, M]

    def chunk_v(pts):  # [L, 3] -> [MB, VW] bf16, col 4*c+f = [pts|1][c*MB+p, f]
        v = np.concatenate([pts, np.ones((pts.shape[0], 1), np.float32)], axis=1)
        flat = v.reshape(-1, MB, 4).transpose(1, 0, 2).reshape(MB, -1)
        out = np.zeros((MB, VW), np.float32)
        out[:, : flat.shape[1]] = flat
        return np.ascontiguousarray(out.astype(np.float16))

    def side(L, R, bias_m, shift_n, vpts, sl):
        ones_m = np.ones((1, L.shape[1]), np.float16)
        ones_n = np.ones((1, H), np.float16)
        bh, bl = _hi_lo16((C0 - bias_m) * 0.5)
        shift = (-0.5 * shift_n[sl]).astype(np.float16)
        return {
            "lhs": np.ascontiguousarray(
                np.vstack([L.astype(np.float16), ones_m, bh[None, :], bl[None, :]])
            ),
            "rhs": np.ascontiguousarray(
                np.vstack([R[:, sl].astype(np.float16), shift[None, :], ones_n, ones_n])
            ),
            "v": chunk_v(vpts),
        }

    in_maps = []
    for c in range(NCORES):
        b, h = divmod(c, 2)
        sl = slice(h * H, (h + 1) * H)
        A = side(tgt_f[b], src_f[b], bb[b], aa[b], tgt[b], sl)
        Bs = side(src_f[b], tgt_f[b], aa[b], bb[b], src[b], sl)
        m = {k + "A": v for k, v in A.items()}
        m.update({k + "B": v for k, v in Bs.items()})
        in_maps.append(m)
    return in_maps


def run(inputs, trace=False, **kw):
    global _PROG
    if _PROG is None:
        _PROG = _build()
    in_maps = _prep_inputs(
        inputs["src"], inputs["tgt"], inputs["src_f"], inputs["tgt_f"]
    )
    bkr = run_bass_kernel_spmd(
        _PROG, in_maps, core_ids=list(range(NCORES)), trace=trace, **kw
    )
    src_corr = np.zeros((B, N, 3), np.float32)
    tgt_corr = np.zeros((B, M, 3), np.float32)
    for c in range(NCORES):
        b, h = divmod(c, 2)
        sl = slice(h * H, (h + 1) * H)
        src_corr[b, sl] = bkr.results[c]["out_src"]
        tgt_corr[b, sl] = bkr.results[c]["out_tgt"]
    return (src_corr, tgt_corr), bkr


def kernel(**inputs):
    out, _ = run(inputs)
    return out
